# revision 48
# baseline (speedup 1.0000x reference)
"""Trainium2 Bass kernel: causal depthwise Conv1d (K=4) + SiLU.

Reference computation (B=4, S=4096, D=2048):
    y[b, s, d] = silu( sum_k w[d, 0, k] * x[b, s-3+k, d] )   (zero-padded left)

Strategy:
  * Host: transpose x to channel-major (D, B, S), left-pad each row with
    4 zeros (row length 4100), cast to bf16, shard D across the 8
    NeuronCores (256 channels each).  Depthwise conv is channel-independent
    -> no inter-core communication.
  * Core: 8 tiles of [128, 4100].  Measured engine rates (NTFF):
      PE:  one 512-col matmul per ~218ns (LDW hidden)  -> 1.71 ns/col
      DVE: 4 tensor_scalar @0.27ns/col + 3 adds @0.53ns/col -> 2.67 ns/col
      ACT: silu ~0.95 ns/col + ~170ns/inst
      HBM: ~412 GB/s aggregate with a ~12us ramp; 17.05 MB -> 41.4us floor
    PE computes tiles 0,2,4,6,7 and the tail of 5; DVE computes 1,3 and
    the head of 5 (input DMAs stream tiles in order, feeding both engines
    in lockstep).  Emission order comes from a causal discrete-event
    co-simulation of PE/DVE/ACT + PSUM-buffer feedback, so the strict-FIFO
    ACT queue (the PSUM drain path) never head-of-line blocks.
  * gpsimd tensorops are avoided entirely: they run ~4x slower than DVE
    and poison concurrent DVE throughput via SBUF port contention.
  * ALL DMAs (input and output) ride the single sync HWDGE FIFO queue:
    transfers drain strictly input-first at full HBM rate (~410 GB/s, no
    input/output bandwidth racing), outputs stream right behind from
    SBUF-buffered y tiles (yp bufs=12).  No SWDGE -> short completion
    receipts at the tail.
  * Host: gather, transpose back, cast to f32.
"""

import os
import sys

sys.path.insert(0, "/opt/trn_rl_repo")

import numpy as np
import ml_dtypes

N_CORES = 8
B, S, D = 4, 4096, 2048
K = 4
PAD = 4
ROW = S + PAD  # 4100
D_LOCAL = D // N_CORES  # 256
G = D_LOCAL // 128  # 2 partition groups per core

MM_N = 512  # PSUM bank limit (512 f32)
WARMUP_MMS = int(os.environ.get("KERNEL_WARMUP", "6"))
PE_CHUNK = int(os.environ.get("KERNEL_PE_CHUNK", "2048"))
PS_BUFS = int(os.environ.get("KERNEL_PS_BUFS", "2"))
# columns of tiles 5/7 computed on DVE (rest on PE)
SPLIT5_DVE = int(os.environ.get("KERNEL_SPLIT5_DVE", "2048"))
SPLIT7_DVE = int(os.environ.get("KERNEL_SPLIT7_DVE", "1536"))
# sim-time threshold (ns) after which outputs use the sync HWDGE queue
HW_OUT_NS = float(os.environ.get("KERNEL_HW_OUT_NS", "34000"))
DVE_TILES = (1, 3, 5)

_CACHE = {}

# ---- measured cost model (ns) for the emission-order simulation ----------
PE_FIRST = 8600.0  # earliest real matmul (after warmup + first chunk)
DVE_FIRST = 10200.0
ACT_FIRST = 12000.0
PE_NS_PER_COL = 1.71
DVE_TS_NS = lambda w: w * 0.27 + 175.0
DVE_TT_NS = lambda w: w * 0.55 + 110.0
ACT_NS = lambda w: w * 0.97 + 180.0


def _dve_chain_ns(w):
    return 4 * DVE_TS_NS(w) + 3 * DVE_TT_NS(w)


def _plan():
    """Simulate the pipeline and return the emission plan:
    (pe_units, dve_units, act_order, out_order, sim_end).
    act_order entries: (ti, lo, hi, kind, t_ready); out entries likewise."""
    # ---- input stream model -------------------------------------------
    # order: wt, mask, t1c0, t0c0, t0c1, t1c1, t0c2, t1c2, then tiles 2..7
    # in two halves each.  HBM input rate ramps 130 -> 410 GB/s over
    # ~11us, and is derated 45% once outputs are in flight (~22us).
    tile_bounds = {0: [0, 1028, 2052, ROW], 1: [0, 1028, 2052, ROW]}
    for ti in range(2, 7):
        tile_bounds[ti] = [0, 2052, ROW]
    tile_bounds[7] = [0, 1540, 2820, ROW]
    dma_order = [(1, 0), (0, 0), (0, 1), (1, 1), (0, 2), (1, 2)]
    for ti in range(2, 7):
        dma_order += [(ti, 0), (ti, 1)]
    dma_order += [(7, 0), (7, 1), (7, 2)]

    arr = {"wt": 8400.0, "mask": 8400.0}  # delivered via the scalar queue
    t = 7700.0
    for item in dma_order:
        if True:
            ti, ci = item
            c0, c1 = tile_bounds[ti][ci], tile_bounds[ti][ci + 1]
            nbytes = (c1 - c0) * 128 * 2
        # outputs ride the SAME sync FIFO queue behind all inputs, so the
        # input stream never shares HBM with them — only the ramp applies
        done = t
        remaining = float(nbytes)
        while remaining > 0.5:
            rate = min(410.0, 130.0 + (done - 7700.0) * 0.028)  # bytes/ns
            step = min(remaining / rate, 500.0)
            done += step
            remaining -= step * rate
        t = done
        arr[(ti, tile_bounds[ti][ci + 1])] = t

    def arrival(ti, col_hi):
        need = min(col_hi + PAD, ROW)
        best = None
        for c1 in tile_bounds[ti][1:]:
            if c1 >= need:
                best = arr[(ti, c1)]
                break
        return best if best is not None else arr[(ti, ROW)]

    # ---- unit lists ---------------------------------------------------
    pe_units = []  # (ti, lo, hi)
    for ti in (0, 2, 4, 6, 7):
        if ti == 0:
            pe_units += [(0, 0, 1024), (0, 1024, 2048), (0, 2048, 3072), (0, 3072, S)]
        elif ti == 7:
            c0 = SPLIT7_DVE
            while c0 < S:
                c1 = min(c0 + PE_CHUNK, S)
                if c1 - c0 > PE_CHUNK // 2 and S - c0 < 2 * PE_CHUNK and S - c1 > 0:
                    c1 = c0 + (S - c0) // 2
                pe_units.append((7, c0, c1))
                c0 = c1
        else:
            for c0 in range(0, S, PE_CHUNK):
                pe_units.append((ti, c0, min(c0 + PE_CHUNK, S)))
        if ti == 4 and SPLIT5_DVE < S:
            for c0 in range(SPLIT5_DVE, S, PE_CHUNK):
                pe_units.append((5, c0, min(c0 + PE_CHUNK, S)))
    dve_units = []  # (ti, lo, hi)
    dve_units += [(1, 0, 1024), (1, 1024, 2048), (1, 2048, S)]
    dve_units += [(3, 0, 2048), (3, 2048, S)]
    if SPLIT5_DVE > 0:
        for c0 in range(0, SPLIT5_DVE, 2048):
            dve_units.append((5, c0, min(c0 + 2048, SPLIT5_DVE)))
    if SPLIT7_DVE > 0:
        dve_units.append((7, 0, SPLIT7_DVE))

    # ---- discrete-event co-simulation ---------------------------------
    # PE/DVE process their unit lists in order; ACT greedily picks the
    # ready silu with the earliest producer-completion.  PSUM feedback:
    # PE unit i waits until unit i-PS_BUFS is fully drained.  DVE c-buffer
    # feedback: chain i waits until chain i-2 is drained.
    # DVE first builds the 8 diag stationaries from the identity mask
    # (8 x 128-col tensor_scalar muls, ~1.7us); PE real work waits on it.
    diag_done = max(arr["mask"], arr["wt"], DVE_FIRST - 2000.0) + 8 * DVE_TS_NS(128)
    pe_done = [None] * len(pe_units)
    dve_done = [None] * len(dve_units)
    silu_done = {}  # (ti, lo) -> ns (all silu chunks of that unit done)
    pe_i = dve_i = 0
    pe_clock, dve_clock, act_clock = PE_FIRST, max(DVE_FIRST, diag_done), ACT_FIRST
    ready = []  # silu candidates: (producer_end, idx_kind, ti, lo, hi)
    act_order = []
    out_order = []

    def silu_chunks(ti, lo, hi):
        return [(c0, min(c0 + 2048, hi)) for c0 in range(lo, hi, 2048)]

    n_work = len(pe_units) + len(dve_units)
    emitted = 0
    guard = 0
    while emitted < n_work or ready:
        guard += 1
        if guard > 10000:
            raise RuntimeError("sim did not converge")
        cands = []
        if pe_i < len(pe_units):
            ti, lo, hi = pe_units[pe_i]
            dep = 0.0
            if pe_i >= PS_BUFS:
                dti, dlo, dhi = pe_units[pe_i - PS_BUFS]
                dep = silu_done.get((dti, dlo), None)
            if dep is not None:
                start = max(pe_clock, arrival(ti, hi), diag_done, dep)
                end = start + (hi - lo) * PE_NS_PER_COL
                cands.append((end, "pe"))
        if dve_i < len(dve_units):
            ti, lo, hi = dve_units[dve_i]
            dep = 0.0
            if dve_i >= 2:
                dti, dlo, dhi = dve_units[dve_i - 2]
                dep = silu_done.get((dti, dlo), None)
            if dep is not None:
                start = max(dve_clock, arrival(ti, hi), arr["wt"], dep)
                end = start + _dve_chain_ns(hi - lo)
                cands.append((end, "dve"))
        if ready:
            ready.sort()
            prod_end, kind, ti, lo, hi = ready[0]
            start = max(act_clock, prod_end)
            dur = sum(ACT_NS(c1 - c0) for c0, c1 in silu_chunks(ti, lo, hi))
            cands.append((start + dur, "act"))
        assert cands, "deadlock in sim"
        end, who = min(cands)
        if who == "pe":
            ti, lo, hi = pe_units[pe_i]
            pe_done[pe_i] = end
            pe_clock = end
            ready.append((end, "pe", ti, lo, hi))
            pe_i += 1
            emitted += 1
        elif who == "dve":
            ti, lo, hi = dve_units[dve_i]
            dve_done[dve_i] = end
            dve_clock = end
            ready.append((end, "dve", ti, lo, hi))
            dve_i += 1
            emitted += 1
        else:
            prod_end, kind, ti, lo, hi = ready.pop(0)
            act_clock = end
            silu_done[(ti, lo)] = end
            act_order.append((ti, lo, hi, kind, end))
            out_order.append((ti, lo, hi, end))
    sim_end = max(act_clock, pe_clock, dve_clock)
    return pe_units, dve_units, act_order, out_order, sim_end


def _build():
    import concourse.tile as tile
    from concourse import bacc, mybir

    nc = bacc.Bacc("TRN2", debug=False, enable_asserts=False, num_devices=N_CORES)
    bf16 = mybir.dt.bfloat16
    f32 = mybir.dt.float32

    x_ap = nc.dram_tensor("x", [G, 128, B, ROW], bf16, kind="ExternalInput").ap()
    mask_ap = nc.dram_tensor("mask", [128, 128], bf16, kind="ExternalInput").ap()
    w_ap = nc.dram_tensor("w", [128, G * K], f32, kind="ExternalInput").ap()
    out_ap = nc.dram_tensor("out", [G, 128, B, S], bf16, kind="ExternalOutput").ap()

    pe_units, dve_units, act_order, out_order, sim_end = _plan()
    if os.environ.get("KERNEL_SIM_DEBUG"):
        print(f"sim_end: {sim_end:.0f} ns")
        for ti, lo, hi, kind, end in act_order:
            print(f"  silu t{ti} [{lo},{hi}) {kind} @{end:.0f}")

    with tile.TileContext(nc) as tc:
        with (
            tc.tile_pool(name="wp", bufs=1) as wp,
            tc.tile_pool(name="xp", bufs=8) as xp,
            tc.tile_pool(name="tp", bufs=2) as tp,
            tc.tile_pool(name="cp", bufs=2) as cp,
            tc.tile_pool(name="ps", bufs=PS_BUFS, space="PSUM") as ps,
            tc.tile_pool(name="yp", bufs=int(os.environ.get("KERNEL_Y_BUFS", "12"))) as yp,
        ):
            wd = wp.tile([128, G * K * 128], bf16, tag="wd")
            wt = wp.tile([128, G * K], f32, tag="wt")
            msk = wp.tile([128, 128], bf16, tag="msk")

            def wdiag(g, k):
                c0 = (g * K + k) * 128
                return wd[:, c0 : c0 + 128]

            def wcol(g, k):
                return wt[:, g * K + k : g * K + k + 1]

            # HAM warmup: dummy matmuls on a zeroed stationary keep the PE
            # p-state ramping; gated only on a gpsimd memset; never read.
            if WARMUP_MMS:
                zt = wp.tile([128, MM_N], bf16, tag="zt")
                nc.gpsimd.memset(zt[:], 0)
                warm = ps.tile([128, PE_CHUNK], f32, tag="acc")
                for _ in range(WARMUP_MMS):
                    nc.tensor.matmul(
                        warm[:, 0:MM_N], zt[:, 0:128], zt[:], start=True, stop=True
                    )

            # ---- input DMAs (sync queue, HWDGE) --------------------------
            tile_bounds = {0: [0, 1028, 2052, ROW], 1: [0, 1028, 2052, ROW]}
            for ti in range(2, 7):
                tile_bounds[ti] = [0, 2052, ROW]
            tile_bounds[7] = [0, 1540, 2820, ROW]
            dma_order = [(1, 0), (0, 0), (0, 1), (1, 1), (0, 2), (1, 2)]
            for ti in range(2, 7):
                dma_order += [(ti, 0), (ti, 1)]
            dma_order += [(7, 0), (7, 1), (7, 2)]
            xts = [None] * 8
            for ti in range(8):
                xt = xp.tile([128, ROW], bf16, tag="xt")
                xts[ti] = xt
            # wt/mask ride the (otherwise idle) scalar HWDGE queue so the
            # sync queue's very first issue is the first x chunk
            nc.scalar.dma_start(out=wt[:], in_=w_ap[:])
            nc.scalar.dma_start(out=msk[:], in_=mask_ap[:])
            for item in dma_order:
                if True:
                    ti, ci = item
                    g, b = divmod(ti, B)
                    c0, c1 = tile_bounds[ti][ci], tile_bounds[ti][ci + 1]
                    nc.sync.dma_start(
                        out=xts[ti][:, c0:c1], in_=x_ap[g, :, b, c0:c1]
                    )

            # DVE builds the 8 [128,128] diag stationaries from the identity
            # mask (idle window before its first x chunk arrives)
            for g in range(G):
                for k in range(K):
                    nc.vector.tensor_scalar_mul(wdiag(g, k), msk[:], wcol(g, k))

            # ---- compute emission (per-engine order from the sim) --------
            accs = {}
            cbufs = {}

            def emit_pe(ti, lo, hi):
                g, b = divmod(ti, B)
                xt = xts[ti]
                cw = hi - lo
                acc = ps.tile([128, cw], f32, tag="acc")
                accs[(ti, lo)] = acc
                for k in range(K):
                    for n0 in range(0, cw, MM_N):
                        xlo = lo + n0 + 1 + k
                        nw = min(MM_N, cw - n0)
                        nc.tensor.matmul(
                            acc[:, n0 : n0 + nw],
                            wdiag(g, k),
                            xt[:, xlo : xlo + nw],
                            start=(k == 0),
                            stop=(k == K - 1),
                        )

            def emit_dve_chain(ti, lo, hi):
                g, b = divmod(ti, B)
                xt = xts[ti]
                W = hi - lo
                t0 = tp.tile([128, W], bf16, tag="t0")
                nc.vector.tensor_scalar_mul(t0[:], xt[:, lo + 1 : lo + 1 + W], wcol(g, 0))
                t1 = tp.tile([128, W], bf16, tag="t1")
                nc.vector.tensor_scalar_mul(t1[:], xt[:, lo + 2 : lo + 2 + W], wcol(g, 1))
                p0 = cp.tile([128, W], bf16, tag="p0")
                nc.vector.tensor_add(p0[:], t0[:], t1[:])
                t2 = tp.tile([128, W], bf16, tag="t0")
                nc.vector.tensor_scalar_mul(t2[:], xt[:, lo + 3 : lo + 3 + W], wcol(g, 2))
                t3 = tp.tile([128, W], bf16, tag="t1")
                nc.vector.tensor_scalar_mul(t3[:], xt[:, lo + 4 : lo + 4 + W], wcol(g, 3))
                p1 = cp.tile([128, W], bf16, tag="p1")
                nc.vector.tensor_add(p1[:], t2[:], t3[:])
                c = cp.tile([128, W], bf16, tag="c")
                nc.vector.tensor_add(c[:], p0[:], p1[:])
                cbufs[(ti, lo)] = c

            # Emit engine streams interleaved in the sim's ACT order: each
            # silu is emitted right after its producer unit (and any earlier
            # units of that engine), so every queue sees the simulated order.
            cursor = {"pe": 0, "dve": 0}
            last_t = act_order[-1][4]
            for ti, lo, hi, kind, end in act_order:
                # emit any compute units up to and including this producer
                if kind == "pe":
                    tgt = pe_units.index((ti, lo, hi))
                    while cursor["pe"] <= tgt:
                        emit_pe(*pe_units[cursor["pe"]])
                        cursor["pe"] += 1
                else:
                    tgt = dve_units.index((ti, lo, hi))
                    while cursor["dve"] <= tgt:
                        emit_dve_chain(*dve_units[cursor["dve"]])
                        cursor["dve"] += 1
                # silu + output
                g, b = divmod(ti, B)
                is_last = end >= last_t - 2000.0
                for c0 in range(lo, hi, 2048):
                    c1 = min(c0 + 2048, hi)
                    W = c1 - c0
                    y = yp.tile([128, W], bf16, tag="y")
                    sw = 1024 if (is_last and W > 1024) else W
                    for s0 in range(0, W, sw):
                        scw = min(sw, W - s0)
                        if kind == "pe":
                            src = accs[(ti, lo)]
                            o = (c0 - lo) + s0
                        else:
                            src = cbufs[(ti, lo)]
                            o = (c0 - lo) + s0
                        nc.scalar.activation(
                            out=y[:, s0 : s0 + scw],
                            in_=src[:, o : o + scw],
                            func=mybir.ActivationFunctionType.Silu,
                        )
                        if is_last:
                            nc.sync.dma_start(
                                out=out_ap[g, :, b, c0 + s0 : c0 + s0 + scw],
                                in_=y[:, s0 : s0 + scw],
                            )
                    if not is_last:
                        nc.sync.dma_start(out=out_ap[g, :, b, c0:c1], in_=y[:])
            # any stragglers (shouldn't happen)
            while cursor["pe"] < len(pe_units):
                emit_pe(*pe_units[cursor["pe"]])
                cursor["pe"] += 1
            while cursor["dve"] < len(dve_units):
                emit_dve_chain(*dve_units[cursor["dve"]])
                cursor["dve"] += 1

    nc.compile()
    return nc


def _get_nc():
    if "nc" not in _CACHE:
        _CACHE["nc"] = _build()
    return _CACHE["nc"]


def _make_in_maps(x, w):
    x = np.asarray(x, dtype=np.float32)
    w = np.asarray(w, dtype=np.float32)

    # (B, S, D) -> (D, B, S), bf16, left-pad rows with PAD zeros.
    x_t = np.ascontiguousarray(x.transpose(2, 0, 1)).astype(ml_dtypes.bfloat16)
    x_pad = np.zeros((D, B, ROW), dtype=ml_dtypes.bfloat16)
    x_pad[:, :, PAD:] = x_t
    w_flat = np.ascontiguousarray(w[:, 0, :])  # (D, K) f32

    eye = np.ascontiguousarray(np.eye(128, dtype=ml_dtypes.bfloat16))
    in_maps = []
    for i in range(N_CORES):
        lo, hi = i * D_LOCAL, (i + 1) * D_LOCAL
        m = {"x": np.ascontiguousarray(x_pad[lo:hi].reshape(G, 128, B, ROW))}
        m["w"] = np.ascontiguousarray(
            w_flat[lo:hi].reshape(G, 128, K).transpose(1, 0, 2).reshape(128, G * K)
        )
        m["mask"] = eye
        in_maps.append(m)
    return in_maps


def _assemble(results):
    parts = []
    for r in results:
        y = np.asarray(r["out"]).reshape(D_LOCAL, B, S)
        parts.append(y)
    y_full = np.concatenate(parts, axis=0)  # (D, B, S) bf16
    return np.ascontiguousarray(y_full.transpose(1, 2, 0)).astype(np.float32)


def kernel(x, w):
    from concourse.bass_utils import run_bass_kernel_spmd

    nc = _get_nc()
    in_maps = _make_in_maps(x, w)
    trace = bool(int(os.environ.get("KERNEL_TRACE", "0")))
    res = None
    err = None
    for attempt in range(3):
        try:
            res = run_bass_kernel_spmd(
                nc, in_maps, core_ids=list(range(N_CORES)),
                trace=trace and attempt == 0,
            )
            break
        except Exception as e:  # transient NRT device errors / missing trace hook
            err = e
            os.environ["BASS_NEVER_TRACE"] = "1"
            trace = False
    if res is None:
        raise err
    _CACHE["last_results"] = res
    return _assemble(res.results)
